# revision 27
# baseline (speedup 1.0000x reference)
"""Trainium2 Bass kernel for GQA attention layer (B=1, S=2048, H=4096,
32 Q heads / 8 KV heads, head_dim 128, RoPE with arbitrary tables).

Sharding: tensor-parallel over heads across 8 NeuronCores — core c gets
Q heads 4c..4c+3 and KV head c (Wq/Wk/Wv column shards, Wo row shard).
Each core computes its partial o_proj output [2048, 4096]; the host sums
the 8 partials (equivalent of the all-reduce).

Schedule: a single software pipeline over q-ranges. Section qr emits the
attention inner loop for q-range qr (scores -> exp -> PV, flash-style,
kt-pair PSUM tiles so one ACT exp covers 1024 columns) interleaved with
the QKV projection + RoPE matmuls of q-range qr+1, so the ACT engine's
exp throughput hides entirely under the PE-bound projection stream. The
last section (qr=3) has no projection work left, so o_proj matmuls of
completed q-ranges fill the PE gaps instead; the remainder drains after,
rotating accumulators across all 8 PSUM banks.

Other specifics:
  - bulk input DMAs ride the sync-engine HW queue in first-use order
    (wk/hst chunked so the first matmuls start ~1MB in); RoPE
    rotate-half swaps and output stores ride the scalar-engine HW queue
    so they never sit behind multi-MB loads.
  - softmax denominator: probs pairs folded on DVE into an f32
    accumulator, partition-reduced with a single fp32r matmul (1
    cycle/row vs 4 for plain fp32), then fast-reciprocal + gpsimd
    partition_broadcast + DVE multiply normalize the PV accumulator.
  - diagonal k-tiles narrow their scores/exp/PV to the unmasked column
    range plus one triangular 128-col mask multiply.
"""

import sys
from contextlib import ExitStack

sys.path.insert(0, "/opt/trn_rl_repo")

import numpy as np
import ml_dtypes

import concourse.bass as bass
import concourse.bacc as bacc
import concourse.mybir as mybir
import concourse.tile as tile
from concourse.bass_utils import run_bass_kernel_spmd
from concourse.masks import make_identity

BF16 = mybir.dt.bfloat16
F32 = mybir.dt.float32
F32R = mybir.dt.float32r

N_CORES = 8
S = 2048
HID = 4096
D = 128
NQ = 4  # q heads per core
KC = HID // 128  # 32 hidden-dim chunks
NQR = S // 512  # 4 q ranges of 512
NST = S // 128  # 16 s-tiles of 128
NHO = HID // 512  # 8 output column tiles of 512
SCALE = 1.0 / float(np.sqrt(D))

_CACHE: dict = {}


def _build_nc():
    nc = bacc.Bacc(None, target_bir_lowering=False, debug=False)

    hst_d = nc.dram_tensor("hst", [NQR, 128, KC, 512], BF16, kind="ExternalInput")
    wq_d = nc.dram_tensor("wq", [NQ, 128, KC, D], BF16, kind="ExternalInput")
    wk_d = nc.dram_tensor("wk", [128, KC, D], BF16, kind="ExternalInput")
    wv_d = nc.dram_tensor("wv", [128, KC, D], BF16, kind="ExternalInput")
    wo_d = nc.dram_tensor("wo", [128, NQ, HID], BF16, kind="ExternalInput")
    cos_d = nc.dram_tensor("cos2", [128, S], F32, kind="ExternalInput")
    sin_d = nc.dram_tensor("sin2", [128, S], F32, kind="ExternalInput")
    out_d = nc.dram_tensor("out", [S, HID], BF16, kind="ExternalOutput")

    with tile.TileContext(nc) as tc, ExitStack() as stack:
        # ---- persistent SBUF pools ----
        const = stack.enter_context(tc.tile_pool(name="const", bufs=1))
        act = stack.enter_context(tc.tile_pool(name="act", bufs=1))
        qt_sb = [
            act.tile([128, S], BF16, tag=f"qt{h}", name=f"qt{h}") for h in range(NQ)
        ]
        kt_sb = act.tile([128, S], BF16, tag="kt")
        vt_sb = act.tile([128, S], BF16, tag="vt")
        v_sb = act.tile([128, NST, 128], BF16, tag="v")  # [s,d] chunks per k-tile
        attn_sb = [
            act.tile([128, S], BF16, tag=f"attn{h}", name=f"attn{h}")
            for h in range(NQ)
        ]
        wqkv = stack.enter_context(tc.tile_pool(name="wqkv", bufs=1))
        hstp = stack.enter_context(tc.tile_pool(name="hstp", bufs=2))
        rope = stack.enter_context(tc.tile_pool(name="rope", bufs=2))
        probs_p = stack.enter_context(tc.tile_pool(name="probs", bufs=3))
        den_p = stack.enter_context(tc.tile_pool(name="den", bufs=2))
        bcast_p = stack.enter_context(tc.tile_pool(name="bcast", bufs=2))
        ostage = stack.enter_context(tc.tile_pool(name="ostage", bufs=4))

        # ---- PSUM pools: 2 + 4 + 2 = 8 banks ----
        psA = stack.enter_context(tc.tile_pool(name="psA", bufs=2, space="PSUM"))
        psS = stack.enter_context(tc.tile_pool(name="psS", bufs=2, space="PSUM"))
        psO = stack.enter_context(tc.tile_pool(name="psO", bufs=2, space="PSUM"))

        # ================= prologue DMAs =================
        # Two HW rings, both loaded in first-use order and chunked <=1MB
        # so small mid-pipeline transfers (rope swaps, outputs) never sit
        # behind a multi-MB bulk load.
        #   sync ring:   wk, hst0, hst1 (hst2/hst3/wo stream in later)
        #   scalar ring: cos/sin (qr0 slice first), wq, wv
        def hst_chunks(dst, qr):
            for r in range(4):
                nc.sync.dma_start(
                    dst[:, r * 8 : (r + 1) * 8, :],
                    hst_d[qr, :, r * 8 : (r + 1) * 8, :],
                )

        # Each ring sustains only ~210GB/s, so the two rings are loaded
        # in parallel, interleaved by consumption time:
        #   scalar: wk, cos/sin[qr0], hst0 r1, r3, wq0, wq2, cos/sin rest
        #   sync:   hst0 r0, r2, wv, wq1, wq3, hst1
        def hst0_r(r, eng):
            eng.dma_start(
                hst_t0[:, r * 8 : (r + 1) * 8, :],
                hst_d[0, :, r * 8 : (r + 1) * 8, :],
            )

        hst_tiles: list = [None] * NQR
        hst_t0 = hstp.tile([128, KC, 512], BF16, tag="hst", name="hst0")
        hst_tiles[0] = hst_t0
        wk_sb = wqkv.tile([128, KC, D], BF16)
        wv_sb = wqkv.tile([128, KC, D], BF16)
        wq_sb = [
            wqkv.tile([128, KC, D], BF16, tag=f"wq{h}", name=f"wq{h}")
            for h in range(NQ)
        ]
        cos_sb = const.tile([128, S], F32)
        sin_sb = const.tile([128, S], F32)

        # balance the k-critical 5MB so both rings finish it ~simultaneously
        # (sync starts ~2.5us earlier); tiny leading chunks let the first
        # matmuls start at ~14us instead of ~19us
        nc.sync.dma_start(wk_sb[:, 0:8, :], wk_d[:, 0:8, :])
        nc.sync.dma_start(hst_t0[:, 0:8, :], hst_d[0, :, 0:8, :])
        nc.scalar.dma_start(wk_sb[:, 8:KC, :], wk_d[:, 8:KC, :])
        hst0_r(2, nc.sync)
        hst0_r(1, nc.scalar)
        hst0_r(3, nc.scalar)
        nc.sync.dma_start(wv_sb[:], wv_d[:])
        nc.scalar.dma_start(cos_sb[:, 0:512], cos_d[:, 0:512])
        nc.scalar.dma_start(sin_sb[:, 0:512], sin_d[:, 0:512])
        nc.sync.dma_start(wq_sb[0][:], wq_d[0])
        nc.scalar.dma_start(wq_sb[1][:], wq_d[1])
        nc.sync.dma_start(wq_sb[2][:], wq_d[2])
        nc.scalar.dma_start(wq_sb[3][:], wq_d[3])
        nc.scalar.dma_start(cos_sb[:, 512:S], cos_d[:, 512:S])
        nc.scalar.dma_start(sin_sb[:, 512:S], sin_d[:, 512:S])
        hst_t1 = hstp.tile([128, KC, 512], BF16, tag="hst", name="hst1")
        hst_tiles[1] = hst_t1
        hst_chunks(hst_t1, 1)
        # wo is allocated late, into hst2's hstp slot (dead once A(2) is
        # emitted) — SBUF is too tight to hold both for the whole kernel.
        wo_ref: dict = {}

        # ---- gpsimd-built constants ----
        identity = const.tile([128, 128], BF16)
        make_identity(nc, identity[:])
        ones_b = const.tile([128, 1], BF16)
        nc.gpsimd.memset(ones_b[:], 1.0)
        # triangular mask for the diagonal 128x128 subtile: rows are k,
        # cols are q; keep q >= k.
        tri = const.tile([128, 128], BF16)
        nc.gpsimd.memset(tri[:], 1.0)
        nc.gpsimd.affine_select(
            out=tri[:],
            in_=tri[:],
            pattern=[[1, 128]],
            compare_op=mybir.AluOpType.is_ge,
            fill=0.0,
            base=0,
            channel_multiplier=-1,
        )

        # ================= A-phase building blocks =================
        def rope_evict(ps, dst_tile, qr):
            """dst[0:64]  = x0*cos - x1*sin
            dst[64:128] = x1*cos + x0*sin   (x0=ps[0:64], x1=ps[64:128])"""
            sl = slice(qr * 512, (qr + 1) * 512)
            raw = rope.tile([128, 512], F32, tag="raw")
            nc.vector.tensor_copy(raw[:], ps[:])
            swp = rope.tile([128, 512], F32, tag="swp")
            # sync ring: bulk loads there are chunked <=1MB, so worst-case
            # queueing is ~3us — and no ACT-queue priority inversion.
            nc.sync.dma_start(swp[0:64, :], raw[64:128, :])
            nc.sync.dma_start(swp[64:128, :], raw[0:64, :])
            nc.vector.tensor_mul(raw[:], raw[:], cos_sb[:, sl])
            nc.vector.tensor_mul(swp[:], swp[:], sin_sb[:, sl])
            nc.vector.tensor_sub(dst_tile[0:64, sl], raw[0:64, :], swp[0:64, :])
            nc.vector.tensor_add(dst_tile[64:128, sl], raw[64:128, :], swp[64:128, :])

        def a_units(qr):
            """Yield thunks for A(qr): 6 projection jobs in c-chunks of 8
            matmuls, evictions, and the v transposes for this qr."""
            hst_t = hst_tiles[qr]
            jobs = [("k", 0), ("v", 0)] + [("q", h) for h in range(NQ)]
            state: dict = {}

            for kind, h in jobs:

                def alloc(kind=kind, h=h):
                    state["ps"] = psA.tile(
                        [128, 512], F32, tag="a", name=f"a{qr}_{kind}{h}"
                    )

                for cg in range(4):

                    def chunk(kind=kind, h=h, cg=cg, alloc=alloc):
                        if cg == 0:
                            alloc(kind, h)
                        ps = state["ps"]
                        for c in range(cg * 8, cg * 8 + 8):
                            if kind == "q":
                                lhsT = wq_sb[h][:, c, :]
                            elif kind == "k":
                                lhsT = wk_sb[:, c, :]
                            else:
                                lhsT = wv_sb[:, c, :]
                            nc.tensor.matmul(
                                ps[:],
                                lhsT,
                                hst_t[:, c, :],
                                start=(c == 0),
                                stop=(c == KC - 1),
                                skip_group_check=True,
                            )

                    yield ("pe", chunk)

                def evict(kind=kind, h=h):
                    ps = state["ps"]
                    if kind == "q":
                        rope_evict(ps, qt_sb[h], qr)
                    elif kind == "k":
                        rope_evict(ps, kt_sb, qr)
                    else:
                        sl = slice(qr * 512, (qr + 1) * 512)
                        nc.vector.tensor_copy(vt_sb[:, sl], ps[:])

                yield ("dve", evict)

                if kind == "v":
                    # transpose this qr's v slice into [s, d] chunks
                    for kt in range(qr * 4, qr * 4 + 4):

                        def transp(kt=kt):
                            pst = psA.tile(
                                [128, 128], BF16, tag="a", name=f"vt{kt}"
                            )
                            nc.tensor.transpose(
                                pst[:],
                                vt_sb[:, kt * 128 : (kt + 1) * 128],
                                identity[:],
                            )
                            nc.vector.tensor_copy(v_sb[:, kt, :], pst[:])

                        yield ("pe", transp)

        # ================= o_proj (phase C) machinery =================
        def c_units():
            for qrC in range(NQR):
                for st in range(qrC * 4, qrC * 4 + 4):
                    for ho in range(NHO):
                        yield ("alloc", qrC, st, ho)
                        for h in range(NQ):
                            yield ("mm", qrC, st, ho, h)
                        yield ("evict", qrC, st, ho)

        c_state = {"gen": c_units(), "pending": None, "tile": None, "nalloc": 0,
                   "nevict": 0}

        def c_alloc_tile(st, ho, wide):
            """Rotate accumulators over psA only (fill mode) or all three
            PSUM pools (drain mode)."""
            i = c_state["nalloc"]
            c_state["nalloc"] += 1
            if not wide:
                return psA.tile([128, 512], F32, tag="a", name=f"c{st}_{ho}"), 512
            # drain mode: rotate over all 8 banks (2 per pool tag, the
            # "s" tiles are 2 banks each) so bank reuse is ~3 units out
            which = i % 6
            if which in (0, 3):
                return (
                    psA.tile([128, 512], F32, tag="a", name=f"c{st}_{ho}"),
                    512,
                )
            if which in (1, 4):
                return (
                    psO.tile([128, 512], F32, tag="o", name=f"c{st}_{ho}"),
                    512,
                )
            return psS.tile([128, 1024], F32, tag="s", name=f"c{st}_{ho}"), 1024

        def emit_c(n_mms, qr_done, wide=False):
            emitted = 0
            while emitted < n_mms:
                unit = c_state["pending"] or next(c_state["gen"], None)
                c_state["pending"] = None
                if unit is None:
                    return False
                if unit[1] > qr_done:
                    c_state["pending"] = unit
                    return False
                if unit[0] == "alloc":
                    _, _, st, ho = unit
                    c_state["tile"], _ = c_alloc_tile(st, ho, wide)
                elif unit[0] == "mm":
                    _, _, st, ho, h = unit
                    nc.tensor.matmul(
                        c_state["tile"][:, 0:512],
                        attn_sb[h][:, st * 128 : (st + 1) * 128],
                        wo_ref["wo"][:, h, ho * 512 : (ho + 1) * 512],
                        start=(h == 0),
                        stop=(h == NQ - 1),
                        skip_group_check=True,
                    )
                    emitted += 1
                else:
                    _, _, st, ho = unit
                    i = c_state["nevict"]
                    c_state["nevict"] += 1
                    stg = ostage.tile([128, 512], BF16, tag="stg")
                    if not wide or i % 2 == 0:
                        # fill mode keeps ACT free — it paces the B(3) exps
                        nc.vector.tensor_copy(stg[:], c_state["tile"][:, 0:512])
                    else:
                        nc.scalar.copy(stg[:], c_state["tile"][:, 0:512])
                    # outputs ride the sync HW queue (inputs are done by now)
                    nc.sync.dma_start(
                        out_d[
                            st * 128 : (st + 1) * 128,
                            ho * 512 : (ho + 1) * 512,
                        ],
                        stg[:],
                    )
            return True

        # ================= B-phase building blocks =================
        def b_units(qr):
            """Yield thunks for the attention inner loop of q-range qr.
            Each pair-visit thunk emits: scores prefetch for the next
            pair + exp, then (after the scheduler's filler) PV + folds."""
            n_kt = 4 * (qr + 1)
            n_pair = n_kt // 2
            qsl = slice(qr * 512, (qr + 1) * 512)

            def c0_of(kt):
                p_idx = kt - 4 * qr
                return 128 * p_idx if p_idx > 0 else 0

            for h in range(NQ):
                st8: dict = {}

                def mm_scores_pair(j, h=h, st8=st8):
                    ps_s = psS.tile(
                        [128, 1024], F32, tag="s", name=f"s{qr}_{h}_{j}"
                    )
                    for idx in range(2):
                        kt = 2 * j + idx
                        c0 = c0_of(kt)
                        nc.tensor.matmul(
                            ps_s[:, idx * 512 + c0 : (idx + 1) * 512],
                            kt_sb[:, kt * 128 : (kt + 1) * 128],
                            qt_sb[h][:, qr * 512 + c0 : (qr + 1) * 512],
                            start=True,
                            stop=True,
                            skip_group_check=True,
                        )
                    st8[j] = ps_s

                def act_exp(j, h=h, st8=st8):
                    ps_s = st8[j]
                    pt = probs_p.tile(
                        [128, 1024], BF16, tag="pt", name=f"pt{qr}_{h}_{j}"
                    )
                    if 2 * j + 1 < 4 * qr:
                        nc.scalar.activation(
                            pt[:],
                            ps_s[:],
                            mybir.ActivationFunctionType.Exp,
                            scale=SCALE,
                        )
                    else:
                        for idx in range(2):
                            kt = 2 * j + idx
                            c0 = c0_of(kt)
                            nc.scalar.activation(
                                pt[:, idx * 512 + c0 : (idx + 1) * 512],
                                ps_s[:, idx * 512 + c0 : (idx + 1) * 512],
                                mybir.ActivationFunctionType.Exp,
                                scale=SCALE,
                            )
                            nc.vector.tensor_mul(
                                pt[:, idx * 512 + c0 : idx * 512 + c0 + 128],
                                pt[:, idx * 512 + c0 : idx * 512 + c0 + 128],
                                tri[:],
                            )
                    st8[("pt", j)] = pt

                def pv_folds(j, h=h, st8=st8):
                    pt = st8.pop(("pt", j))
                    ps_o = st8["o"]
                    den_acc = st8["d"]
                    for idx in range(2):
                        kt = 2 * j + idx
                        c0 = c0_of(kt)
                        nc.tensor.matmul(
                            ps_o[:, c0:512],
                            v_sb[:, kt, :],
                            pt[:, idx * 512 + c0 : (idx + 1) * 512],
                            start=(kt == 0),
                            stop=(kt == n_kt - 1),
                            skip_group_check=True,
                        )
                    pts = probs_p.tile(
                        [128, 512], BF16, tag="pts", name=f"pts{qr}_{h}_{j}"
                    )
                    c0a, c0b = c0_of(2 * j), c0_of(2 * j + 1)
                    if c0b > c0a:
                        nc.vector.tensor_copy(pts[:, c0a:c0b], pt[:, c0a:c0b])
                    nc.vector.tensor_add(
                        pts[:, c0b:512],
                        pt[:, c0b:512],
                        pt[:, 512 + c0b : 1024],
                    )
                    if j == 0:
                        nc.vector.tensor_copy(den_acc[:], pts[:])
                    else:
                        nc.vector.tensor_add(
                            den_acc[:, c0a:512],
                            den_acc[:, c0a:512],
                            pts[:, c0a:512],
                        )

                def head_begin(h=h, st8=st8, msp=mm_scores_pair, ae=act_exp):
                    st8["o"] = psO.tile(
                        [128, 512], F32, tag="o", name=f"o{qr}_{h}"
                    )
                    # bf16 accumulator: the DVE folds hit 2x mode and the
                    # final ones-matmul needs no cast.  Costs ~0.5% on the
                    # denominator (sqrt(n_pair) bf16 rounds).
                    st8["d"] = den_p.tile(
                        [128, 512], BF16, tag="da", name=f"da{qr}_{h}"
                    )
                    msp(0)
                    ae(0)

                yield ("pe", head_begin)
                # filler right after the first scores pair: keeps the PE
                # fed while ACT runs exp(0) at head/section starts
                yield ("fill", None)
                for j in range(n_pair):

                    def prefetch(j=j, n_pair=n_pair, msp=mm_scores_pair,
                                 ae=act_exp):
                        if j + 1 < n_pair:
                            msp(j + 1)
                            ae(j + 1)

                    yield ("pe", prefetch)
                    yield ("fill", None)

                    def fin(j=j, pf=pv_folds):
                        pf(j)

                    yield ("pe", fin)

                def head_end(h=h, st8=st8, n_pair=n_pair):
                    ps_s = st8[n_pair - 1]
                    den_acc = st8["d"]
                    # partition-reduce at bf16 matmul speed (1 cycle/row
                    # vs 4 for fp32)
                    nc.tensor.matmul(
                        ps_s[0:1, 0:512],
                        ones_b[:],
                        den_acc[:],
                        start=True,
                        stop=True,
                        skip_group_check=True,
                    )
                    recip = den_p.tile([1, 512], F32, tag="recip", name=f"rc{qr}_{h}")
                    nc.vector.reciprocal_approx_fast(out=recip[:], in_=ps_s[0:1, 0:512])
                    bc = bcast_p.tile([128, 512], F32, tag="bc")
                    nc.gpsimd.partition_broadcast(bc[:], recip[:])
                    nc.vector.tensor_mul(
                        attn_sb[h][:, qsl], st8["o"][:], bc[:]
                    )

                yield ("pe", head_end)

        # ================= the pipeline =================
        # Solid A(0) first (nothing to overlap with yet).
        for kind, thunk in a_units(0):
            thunk()

        for qr in range(NQR):
            if qr + 2 < NQR:
                nxt = hstp.tile(
                    [128, KC, 512], BF16, tag="hst", name=f"hst{qr+2}"
                )
                hst_tiles[qr + 2] = nxt
                hst_chunks(nxt, qr + 2)

            au = list(a_units(qr + 1)) if qr + 1 < NQR else []
            bu = list(b_units(qr))
            n_fill = sum(1 for k, _ in bu if k == "fill")
            ai = 0
            fills_done = 0
            for k, thunk in bu:
                if k == "fill":
                    fills_done += 1
                    if au:
                        # distribute A units evenly over the fill slots
                        target = (len(au) * fills_done) // n_fill
                        while ai < target:
                            au[ai][1]()
                            ai += 1
                    else:
                        emit_c(2, qr - 1, wide=False)
                else:
                    thunk()
            while ai < len(au):
                au[ai][1]()
                ai += 1

            if qr == 1:
                # A(2) fully emitted — hst2's slot is reclaimable for wo
                wo_sb = hstp.tile([128, NQ, HID], BF16, tag="hst", name="wo_sb")
                wo_ref["wo"] = wo_sb
                for h in range(NQ):
                    nc.sync.dma_start(wo_sb[:, h, :], wo_d[:, h, :])

        # ---- drain the remaining o_proj work ----
        while emit_c(4, NQR - 1, wide=True):
            pass

    nc.compile()
    return nc


def _get_nc():
    if "nc" not in _CACHE:
        _CACHE["nc"] = _build_nc()
    return _CACHE["nc"]


def _bf16(x):
    return np.ascontiguousarray(x.astype(ml_dtypes.bfloat16))


def _prep_in_maps(hidden_states, sin_table, cos_table, Wq, Wk, Wv, Wo):
    hs0 = np.asarray(hidden_states, np.float32).reshape(S, HID)
    # hst[qr, p, c, s] = hs0[qr*512 + s, c*128 + p]
    hst = _bf16(hs0.reshape(NQR, 512, KC, 128).transpose(0, 3, 2, 1))
    cosT = np.asarray(cos_table, np.float32).T  # [64, S]
    sinT = np.asarray(sin_table, np.float32).T
    cos2 = np.ascontiguousarray(np.concatenate([cosT, cosT], 0))  # [128, S]
    sin2 = np.ascontiguousarray(np.concatenate([sinT, sinT], 0))
    Wq = np.asarray(Wq, np.float32)
    Wk = np.asarray(Wk, np.float32)
    Wv = np.asarray(Wv, np.float32)
    Wo = np.asarray(Wo, np.float32)

    in_maps = []
    for c in range(N_CORES):
        wq_c = Wq[:, c * 512 : (c + 1) * 512]  # 4 q heads
        wk_c = Wk[:, c * 128 : (c + 1) * 128]  # 1 kv head
        wv_c = Wv[:, c * 128 : (c + 1) * 128]
        wo_c = Wo[c * 512 : (c + 1) * 512, :]  # matching rows
        # wq per-head-major: [h, p, c, d] with element Wq_c[c*128+p, h*128+d]
        wq_l = wq_c.reshape(KC, 128, NQ, D).transpose(2, 1, 0, 3)
        in_maps.append(
            {
                "hst": hst,
                "wq": _bf16(wq_l),
                "wk": _bf16(wk_c.reshape(KC, 128, D).swapaxes(0, 1)),
                "wv": _bf16(wv_c.reshape(KC, 128, D).swapaxes(0, 1)),
                "wo": _bf16(wo_c.reshape(NQ, 128, HID).swapaxes(0, 1)),
                "cos2": cos2,
                "sin2": sin2,
            }
        )
    return in_maps


def run(trace=False, **inputs):
    nc = _get_nc()
    in_maps = _prep_in_maps(**inputs)
    res = run_bass_kernel_spmd(
        nc, in_maps, core_ids=list(range(N_CORES)), trace=trace
    )
    partials = np.stack(
        [np.asarray(res.results[c]["out"], np.float32) for c in range(N_CORES)]
    )
    out = partials.sum(axis=0, dtype=np.float32).reshape(1, S, HID)
    return out, res


def kernel(**inputs):
    out, _ = run(trace=False, **inputs)
    return out


# revision 35
# speedup vs baseline: 1.0326x; 1.0326x over previous
"""Trainium2 Bass kernel for GQA attention layer (B=1, S=2048, H=4096,
32 Q heads / 8 KV heads, head_dim 128, RoPE with arbitrary tables).

Sharding: tensor-parallel over heads across 8 NeuronCores — core c gets
Q heads 4c..4c+3 and KV head c (Wq/Wk/Wv column shards, Wo row shard).
Each core computes its partial o_proj output [2048, 4096]; the host sums
the 8 partials (equivalent of the all-reduce).

Schedule: a single software pipeline over q-ranges. Section qr emits the
attention inner loop for q-range qr (scores -> exp -> PV, flash-style,
kt-pair PSUM tiles so one ACT exp covers 1024 columns) interleaved with
the QKV projection + RoPE matmuls of q-range qr+1, so the ACT engine's
exp throughput hides entirely under the PE-bound projection stream. The
last section (qr=3) has no projection work left, so o_proj matmuls of
completed q-ranges fill the PE gaps instead; the remainder drains after,
rotating accumulators across all 8 PSUM banks.

Other specifics:
  - bulk input DMAs ride the sync-engine HW queue in first-use order
    (wk/hst chunked so the first matmuls start ~1MB in); RoPE
    rotate-half swaps and output stores ride the scalar-engine HW queue
    so they never sit behind multi-MB loads.
  - softmax denominator: probs pairs folded on DVE into an f32
    accumulator, partition-reduced with a single fp32r matmul (1
    cycle/row vs 4 for plain fp32), then fast-reciprocal + gpsimd
    partition_broadcast + DVE multiply normalize the PV accumulator.
  - diagonal k-tiles narrow their scores/exp/PV to the unmasked column
    range plus one triangular 128-col mask multiply.
"""

import sys
from contextlib import ExitStack

sys.path.insert(0, "/opt/trn_rl_repo")

import numpy as np
import ml_dtypes

import concourse.bass as bass
import concourse.bacc as bacc
import concourse.mybir as mybir
import concourse.tile as tile
from concourse.bass_utils import run_bass_kernel_spmd
from concourse.masks import make_identity

BF16 = mybir.dt.bfloat16
F32 = mybir.dt.float32
F32R = mybir.dt.float32r

N_CORES = 8
S = 2048
HID = 4096
D = 128
NQ = 4  # q heads per core
KC = HID // 128  # 32 hidden-dim chunks
NQR = S // 512  # 4 q ranges of 512
NST = S // 128  # 16 s-tiles of 128
NHO = HID // 512  # 8 output column tiles of 512
SCALE = 1.0 / float(np.sqrt(D))

_CACHE: dict = {}


def _build_nc():
    nc = bacc.Bacc(None, target_bir_lowering=False, debug=False)

    hst_d = nc.dram_tensor("hst", [NQR, 128, KC, 512], BF16, kind="ExternalInput")
    wq_d = nc.dram_tensor("wq", [NQ, 128, KC, D], BF16, kind="ExternalInput")
    wk_d = nc.dram_tensor("wk", [128, KC, D], BF16, kind="ExternalInput")
    wv_d = nc.dram_tensor("wv", [128, KC, D], BF16, kind="ExternalInput")
    wo_d = nc.dram_tensor("wo", [128, NQ, HID], BF16, kind="ExternalInput")
    cos_d = nc.dram_tensor("cos2", [128, S], BF16, kind="ExternalInput")
    sin_d = nc.dram_tensor("sin2", [128, S], BF16, kind="ExternalInput")
    out_d = nc.dram_tensor("out", [S, HID], BF16, kind="ExternalOutput")

    with tile.TileContext(nc) as tc, ExitStack() as stack:
        # ---- persistent SBUF pools ----
        const = stack.enter_context(tc.tile_pool(name="const", bufs=1))
        act = stack.enter_context(tc.tile_pool(name="act", bufs=1))
        qt_sb = [
            act.tile([128, S], BF16, tag=f"qt{h}", name=f"qt{h}") for h in range(NQ)
        ]
        kt_sb = act.tile([128, S], BF16, tag="kt")
        vt_sb = act.tile([128, S], BF16, tag="vt")
        v_sb = act.tile([128, NST, 128], BF16, tag="v")  # [s,d] chunks per k-tile
        attn_sb = [
            act.tile([128, S], BF16, tag=f"attn{h}", name=f"attn{h}")
            for h in range(NQ)
        ]
        wqkv = stack.enter_context(tc.tile_pool(name="wqkv", bufs=1))
        hstp = stack.enter_context(tc.tile_pool(name="hstp", bufs=2))
        rope = stack.enter_context(tc.tile_pool(name="rope", bufs=2))
        probs_p = stack.enter_context(tc.tile_pool(name="probs", bufs=3))
        den_p = stack.enter_context(tc.tile_pool(name="den", bufs=2))
        bcast_p = stack.enter_context(tc.tile_pool(name="bcast", bufs=2))
        ostage = stack.enter_context(tc.tile_pool(name="ostage", bufs=6))

        # ---- PSUM pools: 2 + 4 + 2 = 8 banks ----
        psA = stack.enter_context(tc.tile_pool(name="psA", bufs=2, space="PSUM"))
        psS = stack.enter_context(tc.tile_pool(name="psS", bufs=2, space="PSUM"))
        psO = stack.enter_context(tc.tile_pool(name="psO", bufs=2, space="PSUM"))

        # ================= prologue DMAs =================
        # Two HW rings, both loaded in first-use order and chunked <=1MB
        # so small mid-pipeline transfers (rope swaps, outputs) never sit
        # behind a multi-MB bulk load.
        #   sync ring:   wk, hst0, hst1 (hst2/hst3/wo stream in later)
        #   scalar ring: cos/sin (qr0 slice first), wq, wv
        def hst_chunks(dst, qr):
            for r in range(4):
                nc.sync.dma_start(
                    dst[:, r * 8 : (r + 1) * 8, :],
                    hst_d[qr, :, r * 8 : (r + 1) * 8, :],
                )

        # Each ring sustains only ~210GB/s, so the two rings are loaded
        # in parallel, interleaved by consumption time:
        #   scalar: wk, cos/sin[qr0], hst0 r1, r3, wq0, wq2, cos/sin rest
        #   sync:   hst0 r0, r2, wv, wq1, wq3, hst1
        def hst0_r(r, eng):
            eng.dma_start(
                hst_t0[:, r * 8 : (r + 1) * 8, :],
                hst_d[0, :, r * 8 : (r + 1) * 8, :],
            )

        hst_tiles: list = [None] * NQR
        hst_t0 = hstp.tile([128, KC, 512], BF16, tag="hst", name="hst0")
        hst_tiles[0] = hst_t0
        wk_sb = wqkv.tile([128, KC, D], BF16)
        wv_sb = wqkv.tile([128, KC, D], BF16)
        wq_sb = [
            wqkv.tile([128, KC, D], BF16, tag=f"wq{h}", name=f"wq{h}")
            for h in range(NQ)
        ]
        cos_sb = const.tile([128, S], BF16)
        sin_sb = const.tile([128, S], BF16)

        # balance the k-critical 5MB so both rings finish it ~simultaneously
        # (sync starts ~2.5us earlier); tiny leading chunks let the first
        # matmuls start at ~14us instead of ~19us
        nc.sync.dma_start(wk_sb[:, 0:8, :], wk_d[:, 0:8, :])
        nc.sync.dma_start(hst_t0[:, 0:8, :], hst_d[0, :, 0:8, :])
        nc.scalar.dma_start(wk_sb[:, 8:KC, :], wk_d[:, 8:KC, :])
        hst0_r(2, nc.sync)
        hst0_r(1, nc.scalar)
        hst0_r(3, nc.scalar)
        nc.sync.dma_start(wv_sb[:], wv_d[:])
        nc.scalar.dma_start(cos_sb[:, 0:512], cos_d[:, 0:512])
        nc.scalar.dma_start(sin_sb[:, 0:512], sin_d[:, 0:512])
        nc.sync.dma_start(wq_sb[0][:], wq_d[0])
        nc.scalar.dma_start(wq_sb[1][:], wq_d[1])
        nc.sync.dma_start(wq_sb[2][:], wq_d[2])
        nc.scalar.dma_start(wq_sb[3][:], wq_d[3])
        nc.scalar.dma_start(cos_sb[:, 512:S], cos_d[:, 512:S])
        nc.scalar.dma_start(sin_sb[:, 512:S], sin_d[:, 512:S])
        hst_t1 = hstp.tile([128, KC, 512], BF16, tag="hst", name="hst1")
        hst_tiles[1] = hst_t1
        hst_chunks(hst_t1, 1)
        # wo is allocated late, into hst2's hstp slot (dead once A(2) is
        # emitted) — SBUF is too tight to hold both for the whole kernel.
        wo_ref: dict = {}

        # ---- gpsimd-built constants ----
        identity = const.tile([128, 128], BF16)
        make_identity(nc, identity[:])
        ones_b = const.tile([128, 1], BF16)
        nc.gpsimd.memset(ones_b[:], 1.0)
        # triangular mask for the diagonal 128x128 subtile: rows are k,
        # cols are q; keep q >= k.
        tri = const.tile([128, 128], BF16)
        nc.gpsimd.memset(tri[:], 1.0)
        nc.gpsimd.affine_select(
            out=tri[:],
            in_=tri[:],
            pattern=[[1, 128]],
            compare_op=mybir.AluOpType.is_ge,
            fill=0.0,
            base=0,
            channel_multiplier=-1,
        )
        # pswap: permutation matrix swapping partition halves, so the RoPE
        # rotate-half is one PE matmul instead of an SBUF-SBUF DMA (which
        # would queue behind multi-MB loads on the rings)
        pswap = const.tile([128, 128], BF16)
        ptmp = const.tile([128, 128], BF16)
        nc.gpsimd.memset(pswap[:], 1.0)
        nc.gpsimd.memset(ptmp[:], 1.0)
        nc.gpsimd.affine_select(
            out=pswap[:],
            in_=pswap[:],
            pattern=[[1, 128]],
            compare_op=mybir.AluOpType.is_equal,
            fill=0.0,
            base=64,
            channel_multiplier=-1,
        )
        nc.gpsimd.affine_select(
            out=ptmp[:],
            in_=ptmp[:],
            pattern=[[1, 128]],
            compare_op=mybir.AluOpType.is_equal,
            fill=0.0,
            base=-64,
            channel_multiplier=-1,
        )
        nc.gpsimd.tensor_add(pswap[:], pswap[:], ptmp[:])

        # ================= A-phase building blocks =================
        def rope_evict(ps, dst_tile, qr):
            """dst[0:64]  = x0*cos - x1*sin
            dst[64:128] = x1*cos + x0*sin   (x0=ps[0:64], x1=ps[64:128]).
            The rotate-half is a pswap matmul back into the job's own
            (now dead) PSUM accumulator — no DMA, no ring traffic."""
            sl = slice(qr * 512, (qr + 1) * 512)
            raw = rope.tile([128, 512], BF16, tag="raw")
            nc.vector.tensor_copy(raw[:], ps[:])
            nc.tensor.matmul(
                ps[:], pswap[:], raw[:], start=True, stop=True,
                skip_group_check=True,
            )
            m1 = rope.tile([128, 512], BF16, tag="m1")
            nc.vector.tensor_mul(m1[:], raw[:], cos_sb[:, sl])
            m2 = rope.tile([128, 512], BF16, tag="m2")
            nc.vector.tensor_mul(m2[:], ps[:], sin_sb[:, sl])
            nc.vector.tensor_sub(dst_tile[0:64, sl], m1[0:64, :], m2[0:64, :])
            nc.vector.tensor_add(dst_tile[64:128, sl], m1[64:128, :], m2[64:128, :])

        def a_units(qr):
            """Yield thunks for A(qr): 6 projection jobs in c-chunks of 8
            matmuls, evictions, and the v transposes for this qr.  Each
            job's evict is delayed until after the NEXT job's first chunk
            so the rope permute-matmul (which waits on the DVE's PSUM
            read) never bubbles the in-order PE queue."""
            hst_t = hst_tiles[qr]
            jobs = [("k", 0), ("v", 0)] + [("q", h) for h in range(NQ)]
            state: dict = {}
            pending = []

            for kind, h in jobs:

                def alloc(kind=kind, h=h):
                    # ps tiles are keyed per job so the delayed evict
                    # reads the right accumulator
                    state[(kind, h)] = psA.tile(
                        [128, 512], F32, tag="a", name=f"a{qr}_{kind}{h}"
                    )

                for cg in range(4):

                    def chunk(kind=kind, h=h, cg=cg, alloc=alloc):
                        if cg == 0:
                            alloc()
                        ps = state[(kind, h)]
                        for c in range(cg * 8, cg * 8 + 8):
                            if kind == "q":
                                lhsT = wq_sb[h][:, c, :]
                            elif kind == "k":
                                lhsT = wk_sb[:, c, :]
                            else:
                                lhsT = wv_sb[:, c, :]
                            nc.tensor.matmul(
                                ps[:],
                                lhsT,
                                hst_t[:, c, :],
                                start=(c == 0),
                                stop=(c == KC - 1),
                                skip_group_check=True,
                            )

                    yield ("pe", chunk)
                    if cg == 0 and pending:
                        for u in pending:
                            yield u
                        pending = []

                def evict(kind=kind, h=h):
                    ps = state.pop((kind, h))
                    if kind == "q":
                        rope_evict(ps, qt_sb[h], qr)
                    elif kind == "k":
                        rope_evict(ps, kt_sb, qr)
                    else:
                        sl = slice(qr * 512, (qr + 1) * 512)
                        nc.vector.tensor_copy(vt_sb[:, sl], ps[:])

                if kind == "v":
                    # v's evict + transposes stay immediate: the transpose
                    # scratch tiles must chain through both psA slots
                    # before the next job's accumulator is allocated
                    yield ("dve", evict)
                    for kt in range(qr * 4, qr * 4 + 4):

                        def transp(kt=kt):
                            pst = psA.tile(
                                [128, 128], BF16, tag="a", name=f"vt{kt}"
                            )
                            nc.tensor.transpose(
                                pst[:],
                                vt_sb[:, kt * 128 : (kt + 1) * 128],
                                identity[:],
                            )
                            nc.vector.tensor_copy(v_sb[:, kt, :], pst[:])

                        yield ("pe", transp)
                else:
                    pending.append(("dve", evict))

            for u in pending:
                yield u

        # ================= o_proj (phase C) machinery =================
        def c_units():
            for qrC in range(NQR):
                for st in range(qrC * 4, qrC * 4 + 4):
                    for ho in range(NHO):
                        yield ("alloc", qrC, st, ho)
                        for h in range(NQ):
                            yield ("mm", qrC, st, ho, h)
                        yield ("evict", qrC, st, ho)

        c_state = {"gen": c_units(), "pending": None, "tile": None, "nalloc": 0,
                   "nevict": 0}

        def c_alloc_tile(st, ho, wide):
            """Rotate accumulators over psA only (fill mode) or all three
            PSUM pools (drain mode)."""
            i = c_state["nalloc"]
            c_state["nalloc"] += 1
            if not wide:
                return psA.tile([128, 512], F32, tag="a", name=f"c{st}_{ho}"), 512
            # drain mode: rotate over all 8 banks (2 per pool tag, the
            # "s" tiles are 2 banks each) so bank reuse is ~3 units out
            which = i % 6
            if which in (0, 3):
                return (
                    psA.tile([128, 512], F32, tag="a", name=f"c{st}_{ho}"),
                    512,
                )
            if which in (1, 4):
                return (
                    psO.tile([128, 512], F32, tag="o", name=f"c{st}_{ho}"),
                    512,
                )
            return psS.tile([128, 1024], F32, tag="s", name=f"c{st}_{ho}"), 1024

        def emit_c(n_mms, qr_done, wide=False):
            emitted = 0
            while emitted < n_mms:
                unit = c_state["pending"] or next(c_state["gen"], None)
                c_state["pending"] = None
                if unit is None:
                    return False
                if unit[1] > qr_done:
                    c_state["pending"] = unit
                    return False
                if unit[0] == "alloc":
                    _, _, st, ho = unit
                    c_state["tile"], _ = c_alloc_tile(st, ho, wide)
                elif unit[0] == "mm":
                    _, _, st, ho, h = unit
                    nc.tensor.matmul(
                        c_state["tile"][:, 0:512],
                        attn_sb[h][:, st * 128 : (st + 1) * 128],
                        wo_ref["wo"][:, h, ho * 512 : (ho + 1) * 512],
                        start=(h == 0),
                        stop=(h == NQ - 1),
                        skip_group_check=True,
                    )
                    emitted += 1
                else:
                    _, _, st, ho = unit
                    i = c_state["nevict"]
                    c_state["nevict"] += 1
                    stg = ostage.tile([128, 512], BF16, tag="stg")
                    if not wide or i % 2 == 0:
                        # fill mode keeps ACT free — it paces the B(3) exps
                        nc.vector.tensor_copy(stg[:], c_state["tile"][:, 0:512])
                    else:
                        nc.scalar.copy(stg[:], c_state["tile"][:, 0:512])
                    # outputs ride the sync HW queue (inputs are done by now)
                    nc.sync.dma_start(
                        out_d[
                            st * 128 : (st + 1) * 128,
                            ho * 512 : (ho + 1) * 512,
                        ],
                        stg[:],
                    )
            return True

        # ================= B-phase building blocks =================
        def b_units(qr):
            """Yield thunks for the attention inner loop of q-range qr.
            Each pair-visit thunk emits: scores prefetch for the next
            pair + exp, then (after the scheduler's filler) PV + folds."""
            n_kt = 4 * (qr + 1)
            n_pair = n_kt // 2
            qsl = slice(qr * 512, (qr + 1) * 512)

            def c0_of(kt):
                p_idx = kt - 4 * qr
                return 128 * p_idx if p_idx > 0 else 0

            for h in range(NQ):
                st8: dict = {}

                def mm_scores_pair(j, h=h, st8=st8):
                    ps_s = psS.tile(
                        [128, 1024], F32, tag="s", name=f"s{qr}_{h}_{j}"
                    )
                    for idx in range(2):
                        kt = 2 * j + idx
                        c0 = c0_of(kt)
                        nc.tensor.matmul(
                            ps_s[:, idx * 512 + c0 : (idx + 1) * 512],
                            kt_sb[:, kt * 128 : (kt + 1) * 128],
                            qt_sb[h][:, qr * 512 + c0 : (qr + 1) * 512],
                            start=True,
                            stop=True,
                            skip_group_check=True,
                        )
                    st8[j] = ps_s

                def act_exp(j, h=h, st8=st8):
                    ps_s = st8[j]
                    pt = probs_p.tile(
                        [128, 1024], BF16, tag="pt", name=f"pt{qr}_{h}_{j}"
                    )
                    if 2 * j + 1 < 4 * qr:
                        nc.scalar.activation(
                            pt[:],
                            ps_s[:],
                            mybir.ActivationFunctionType.Exp,
                            scale=SCALE,
                        )
                    else:
                        for idx in range(2):
                            kt = 2 * j + idx
                            c0 = c0_of(kt)
                            nc.scalar.activation(
                                pt[:, idx * 512 + c0 : (idx + 1) * 512],
                                ps_s[:, idx * 512 + c0 : (idx + 1) * 512],
                                mybir.ActivationFunctionType.Exp,
                                scale=SCALE,
                            )
                            nc.vector.tensor_mul(
                                pt[:, idx * 512 + c0 : idx * 512 + c0 + 128],
                                pt[:, idx * 512 + c0 : idx * 512 + c0 + 128],
                                tri[:],
                            )
                    st8[("pt", j)] = pt

                def pv_folds(j, h=h, st8=st8):
                    pt = st8.pop(("pt", j))
                    ps_o = st8["o"]
                    den_acc = st8["d"]
                    for idx in range(2):
                        kt = 2 * j + idx
                        c0 = c0_of(kt)
                        nc.tensor.matmul(
                            ps_o[:, c0:512],
                            v_sb[:, kt, :],
                            pt[:, idx * 512 + c0 : (idx + 1) * 512],
                            start=(kt == 0),
                            stop=(kt == n_kt - 1),
                            skip_group_check=True,
                        )
                    pts = probs_p.tile(
                        [128, 512], BF16, tag="pts", name=f"pts{qr}_{h}_{j}"
                    )
                    c0a, c0b = c0_of(2 * j), c0_of(2 * j + 1)
                    if c0b > c0a:
                        nc.vector.tensor_copy(pts[:, c0a:c0b], pt[:, c0a:c0b])
                    nc.vector.tensor_add(
                        pts[:, c0b:512],
                        pt[:, c0b:512],
                        pt[:, 512 + c0b : 1024],
                    )
                    if j == 0:
                        nc.vector.tensor_copy(den_acc[:], pts[:])
                    else:
                        nc.vector.tensor_add(
                            den_acc[:, c0a:512],
                            den_acc[:, c0a:512],
                            pts[:, c0a:512],
                        )

                def head_begin(h=h, st8=st8, msp=mm_scores_pair, ae=act_exp):
                    st8["o"] = psO.tile(
                        [128, 512], F32, tag="o", name=f"o{qr}_{h}"
                    )
                    # bf16 accumulator: the DVE folds hit 2x mode and the
                    # final ones-matmul needs no cast.  Costs ~0.5% on the
                    # denominator (sqrt(n_pair) bf16 rounds).
                    st8["d"] = den_p.tile(
                        [128, 512], BF16, tag="da", name=f"da{qr}_{h}"
                    )
                    msp(0)
                    ae(0)

                yield ("pe", head_begin)
                # filler right after the first scores pair: keeps the PE
                # fed while ACT runs exp(0) at head/section starts
                yield ("fill", None)
                for j in range(n_pair):

                    def prefetch(j=j, n_pair=n_pair, msp=mm_scores_pair,
                                 ae=act_exp):
                        if j + 1 < n_pair:
                            msp(j + 1)
                            ae(j + 1)

                    yield ("pe", prefetch)
                    yield ("fill", None)

                    def fin(j=j, pf=pv_folds):
                        pf(j)

                    yield ("pe", fin)

                def head_end(h=h, st8=st8, n_pair=n_pair):
                    ps_s = st8[n_pair - 1]
                    den_acc = st8["d"]
                    # partition-reduce at bf16 matmul speed (1 cycle/row
                    # vs 4 for fp32)
                    nc.tensor.matmul(
                        ps_s[0:1, 0:512],
                        ones_b[:],
                        den_acc[:],
                        start=True,
                        stop=True,
                        skip_group_check=True,
                    )
                    recip = den_p.tile([1, 512], F32, tag="recip", name=f"rc{qr}_{h}")
                    nc.vector.reciprocal_approx_fast(out=recip[:], in_=ps_s[0:1, 0:512])
                    bc = bcast_p.tile([128, 512], F32, tag="bc")
                    nc.gpsimd.partition_broadcast(bc[:], recip[:])
                    nc.vector.tensor_mul(
                        attn_sb[h][:, qsl], st8["o"][:], bc[:]
                    )

                yield ("pe", head_end)

        # ================= the pipeline =================
        # Solid A(0) first (nothing to overlap with yet).
        for kind, thunk in a_units(0):
            thunk()

        for qr in range(NQR):
            if qr + 2 < NQR:
                nxt = hstp.tile(
                    [128, KC, 512], BF16, tag="hst", name=f"hst{qr+2}"
                )
                hst_tiles[qr + 2] = nxt
                hst_chunks(nxt, qr + 2)

            au = list(a_units(qr + 1)) if qr + 1 < NQR else []
            bu = list(b_units(qr))
            n_fill = sum(1 for k, _ in bu if k == "fill")
            ai = 0
            fills_done = 0
            for k, thunk in bu:
                if k == "fill":
                    fills_done += 1
                    if au:
                        # distribute A units evenly over the fill slots
                        target = (len(au) * fills_done) // n_fill
                        while ai < target:
                            au[ai][1]()
                            ai += 1
                    else:
                        emit_c(2, qr - 1, wide=False)
                else:
                    thunk()
            while ai < len(au):
                au[ai][1]()
                ai += 1

            if qr == 1:
                # A(2) fully emitted — hst2's slot is reclaimable for wo
                wo_sb = hstp.tile([128, NQ, HID], BF16, tag="hst", name="wo_sb")
                wo_ref["wo"] = wo_sb
                for h in range(NQ):
                    nc.sync.dma_start(wo_sb[:, h, :], wo_d[:, h, :])

        # ---- drain the remaining o_proj work ----
        while emit_c(4, NQR - 1, wide=True):
            pass

    nc.compile()
    return nc


def _get_nc():
    if "nc" not in _CACHE:
        _CACHE["nc"] = _build_nc()
    return _CACHE["nc"]


def _bf16(x):
    return np.ascontiguousarray(x.astype(ml_dtypes.bfloat16))


def _prep_in_maps(hidden_states, sin_table, cos_table, Wq, Wk, Wv, Wo):
    hs0 = np.asarray(hidden_states, np.float32).reshape(S, HID)
    # hst[qr, p, c, s] = hs0[qr*512 + s, c*128 + p]
    hst = _bf16(hs0.reshape(NQR, 512, KC, 128).transpose(0, 3, 2, 1))
    cosT = np.asarray(cos_table, np.float32).T  # [64, S]
    sinT = np.asarray(sin_table, np.float32).T
    cos2 = _bf16(np.concatenate([cosT, cosT], 0))  # [128, S]
    sin2 = _bf16(np.concatenate([sinT, sinT], 0))
    Wq = np.asarray(Wq, np.float32)
    Wk = np.asarray(Wk, np.float32)
    Wv = np.asarray(Wv, np.float32)
    Wo = np.asarray(Wo, np.float32)

    in_maps = []
    for c in range(N_CORES):
        wq_c = Wq[:, c * 512 : (c + 1) * 512]  # 4 q heads
        wk_c = Wk[:, c * 128 : (c + 1) * 128]  # 1 kv head
        wv_c = Wv[:, c * 128 : (c + 1) * 128]
        wo_c = Wo[c * 512 : (c + 1) * 512, :]  # matching rows
        # wq per-head-major: [h, p, c, d] with element Wq_c[c*128+p, h*128+d]
        wq_l = wq_c.reshape(KC, 128, NQ, D).transpose(2, 1, 0, 3)
        in_maps.append(
            {
                "hst": hst,
                "wq": _bf16(wq_l),
                "wk": _bf16(wk_c.reshape(KC, 128, D).swapaxes(0, 1)),
                "wv": _bf16(wv_c.reshape(KC, 128, D).swapaxes(0, 1)),
                "wo": _bf16(wo_c.reshape(NQ, 128, HID).swapaxes(0, 1)),
                "cos2": cos2,
                "sin2": sin2,
            }
        )
    return in_maps


def run(trace=False, **inputs):
    nc = _get_nc()
    in_maps = _prep_in_maps(**inputs)
    res = run_bass_kernel_spmd(
        nc, in_maps, core_ids=list(range(N_CORES)), trace=trace
    )
    partials = np.stack(
        [np.asarray(res.results[c]["out"], np.float32) for c in range(N_CORES)]
    )
    out = partials.sum(axis=0, dtype=np.float32).reshape(1, S, HID)
    return out, res


def kernel(**inputs):
    out, _ = run(trace=False, **inputs)
    return out
